# revision 1
# baseline (speedup 1.0000x reference)
"""Bidirectional DSS/Mamba block on 8 trn2 cores (Bass/Tile) — v2.

Sharding: core = (batch b = core//2, d_inner half = core%2), as v1.

Changes vs v1 (all hardware-measured):
- Input loads spread across the sync/scalar/gpsimd DMA queues so the first
  in-proj matmul starts ~5us earlier.
- Softplus exps/adds/lns batched per-stage to cut activation-table
  thrashing (8 -> 5 ACT_TABLE_LOADs); the tail sigmoid is computed as
  1/(1+exp(-q)) under the already-loaded exp table; G2 matvec in bf16.
- B/C broadcast rows prefetched group-wise (4 n at a time, 2 DMAs/group)
  from the DRAM bounce, removing 32 DMAs + semaphore waits from the scan
  loop's dependency chains. Scan-phase tiles live in 16B-aligned segments
  (SEG=904) of grouped buffers.
- Everything elementwise stays on the DVE: the Pool engine shares SBUF
  ports with the DVE, so offloading products to gpsimd slows every DVE op
  15-50% and is net-negative (measured). tensor_tensor_reduce crashes the
  runtime; scans are DVE-only (Pool fails the ISA engine check).
- The pooled-mean AllGather input DMA goes out on the idle gpsimd queue
  (the sync queue had a ~10us backlog at that moment in v1), and ~24
  throwaway matmuls keep the PE p-state hot through the ~35us collective
  wait so the gate matvec + out-proj run at full clock.
"""

import os
import sys

sys.path.insert(0, "/opt/trn_rl_repo")

from contextlib import ExitStack

import ml_dtypes
import numpy as np

import concourse.bass as bass
import concourse.bacc as bacc
import concourse.tile as tile
from concourse import mybir
from concourse.bass_utils import run_bass_kernel_spmd

F32 = mybir.dt.float32
BF16 = mybir.dt.bfloat16
U32 = mybir.dt.uint32
AF = mybir.ActivationFunctionType
OP = mybir.AluOpType

B, L, DM, DS, DI, R = 4, 900, 256, 16, 512, 16
DH = DI // 2          # d_inner channels per core
NDT = DH // 128       # 128-channel tiles per core (2)
FCH = [(0, 512), (512, L - 512)]  # PSUM-bank-aligned L chunks
SEG = 904             # aligned segment stride in grouped tiles (16B)
ONE_F32_BITS = 0x3F800000

EXCH = os.environ.get("K_EXCH", "cc")       # "cc" | "p2p" (pair-HBM is not shared in this runtime)
DAF_BF16 = os.environ.get("K_DAFBF", "0") == "1"
HC_POOL = int(os.environ.get("K_HCPOOL", "0"))  # DVE+Pool share SBUF ports: Pool offload is net-negative
DBU_POOL = os.environ.get("K_DBUPOOL", "0") == "1"
N_POLLS = 12


def _ap(t, dims, extra=0):
    """AP over tile t's buffer with explicit free dims (partition dim kept)."""
    return bass.AP(tensor=t.tensor, offset=t.offset + extra, ap=[t.ap[0]] + dims)


def _seg(t, k, f0=0, fl=L):
    """Slice segment k's [f0:f0+fl] columns of a grouped [128, G*SEG] tile."""
    return t[:, k * SEG + f0 : k * SEG + f0 + fl]


def _build_module(shared_a: bool, exch: str):
    G = 4 if shared_a else 2      # n-group size (SBUF bound when A_f != A_b)
    NG = DS // G
    GW = G * SEG                  # grouped tile width
    DAF_DT = BF16 if DAF_BF16 else F32

    nc = bacc.Bacc("TRN2", num_devices=8)

    ein = lambda n, s, d=F32: nc.dram_tensor(n, s, d, kind="ExternalInput")
    hsT = ein("hsT", [DM, L], BF16)
    WinxT = ein("WinxT", [DM, DI], BF16)
    WinzT = ein("WinzT", [DM, DH], BF16)
    WxT = ein("WxT", [DI, R + 2 * DS], BF16)
    WdtT = ein("WdtT", [R, DH], BF16)
    bdt = ein("bdt", [128, NDT])
    Afc = ein("Afc", [128, NDT * DS])
    Abc = ein("Abc", [128, NDT * DS])
    Ddf = ein("Ddf", [DH, 128], BF16)
    Ddb = ein("Ddb", [DH, 128], BF16)
    I128 = ein("I128", [128, 128], BF16)
    G2T = ein("G2T", [2 * DI, 2 * DH], BF16)
    bgate2 = ein("bgate2", [1, 2 * DH])
    WoT = ein("WoT", [2 * DH, DM], BF16)
    pctl = ein("pctl", [1, 4], U32)      # [parity, ...]
    outp = nc.dram_tensor("outp", [DM, L], F32, kind="ExternalOutput")

    bc_dram = nc.dram_tensor("bc_bounce", [2 * DS, L], BF16, kind="Internal")
    g_dram = nc.dram_tensor("g_dram", [1, 2 * DH], F32, kind="Internal")
    cc_zin = nc.dram_tensor("cc_zin", [1, 4], F32, kind="Internal")
    cc_zout = nc.dram_tensor("cc_zout", [1, 8], F32, kind="Internal")
    if exch == "p2p":
        shm = nc.dram_tensor("shm", [1, 2 * DI], F32, kind="Internal",
                             addr_space="Shared")
        shfl = nc.dram_tensor("shfl", [1, 8], U32, kind="Internal",
                              addr_space="Shared")
    else:
        u_cc_in = nc.dram_tensor("u_cc_in", [1, 2 * DH], F32, kind="Internal")
        u_cc_out = nc.dram_tensor("u_cc_out", [1, 2 * DI], F32, kind="Internal")

    with ExitStack() as ctx:
        tc = ctx.enter_context(tile.TileContext(nc))
        wpool = ctx.enter_context(tc.tile_pool(name="weights", bufs=1))
        apool = ctx.enter_context(tc.tile_pool(name="acts", bufs=1))
        psem = ctx.enter_context(nc.semaphore("p2p_sem"))

        def load(name, dram, p, f, eng=None):
            ts = []
            for i in range(0, p, 128):
                pp = min(128, p - i)
                t = wpool.tile([pp, f], dram.dtype, tag=f"{name}{i}", name=f"{name}{i}")
                (eng or nc.sync).dma_start(out=t, in_=dram[i : i + pp, :])
                ts.append(t)
            return ts

        # ---- input loads, spread across queues (first-needed first) ----
        hs0 = load("hs0", hsT, 128, L)[0]                     # sync
        hs1_t = wpool.tile([128, L], BF16, tag="hs1", name="hs1")
        nc.scalar.dma_start(out=hs1_t, in_=hsT[128:256, :])   # scalar queue
        hs = [hs0, hs1_t]
        winx0 = wpool.tile([128, DI], BF16, tag="winx0", name="winx0")
        nc.sync.dma_start(out=winx0, in_=WinxT[0:128, :])
        winx1 = wpool.tile([128, DI], BF16, tag="winx1", name="winx1")
        nc.scalar.dma_start(out=winx1, in_=WinxT[128:256, :])
        winx = [winx0, winx1]
        winz = load("winz", WinzT, DM, DH, eng=nc.gpsimd)
        wx = load("wx", WxT, DI, R + 2 * DS, eng=nc.gpsimd)
        wdt = load("wdt", WdtT, R, DH, eng=nc.gpsimd)
        bdt_s = load("bdt", bdt, 128, NDT, eng=nc.gpsimd)[0]
        af_s = load("afc", Afc, 128, NDT * DS, eng=nc.gpsimd)[0]
        ab_s = load("abc", Abc, 128, NDT * DS, eng=nc.gpsimd)[0]
        ddf = load("ddf", Ddf, DH, 128, eng=nc.gpsimd)
        ddb = load("ddb", Ddb, DH, 128, eng=nc.gpsimd)
        ident = load("ident", I128, 128, 128, eng=nc.gpsimd)[0]
        pct_s = load("pct", pctl, 1, 4, eng=nc.gpsimd)[0]
        wo = load("wo", WoT, 2 * DH, DM, eng=nc.gpsimd)
        g2 = load("g2", G2T, 2 * DI, 2 * DH, eng=nc.gpsimd)
        bgate_r = load("bgate2", bgate2, 1, 2 * DH, eng=nc.gpsimd)[0]

        # ---- start barrier + flag zeroing (p2p) ----
        zin = apool.tile([1, 4], F32, tag="zin", name="zin")
        nc.vector.memset(zin, 0.0)
        nc.sync.dma_start(out=cc_zin[:, :], in_=zin)
        nc.gpsimd.collective_compute(
            "AllGather", OP.bypass,
            replica_groups=[[0, 1], [2, 3], [4, 5], [6, 7]],
            ins=[cc_zin[:, :]], outs=[cc_zout[:, :]],
        )
        if exch == "p2p":
            zf = apool.tile([1, 8], U32, tag="zf", name="zf")
            nc.sync.dma_start(out=zf, in_=cc_zout[:, :].bitcast(U32))  # zeros, post-barrier
            nc.sync.dma_start(out=shfl[:, :], in_=zf)
        one_sb = apool.tile([1, 4], U32, tag="one", name="one")
        nc.vector.memset(one_sb, 1)

        # ---- in-proj: x full (silu), z own half (silu) ----
        xT = [apool.tile([128, L], BF16, tag=f"xT{i}", name=f"xT{i}") for i in range(4)]
        zg = [apool.tile([128, L], BF16, tag=f"zg{i}", name=f"zg{i}") for i in range(NDT)]
        with tc.tile_pool(name="ps_early", bufs=2, space="PSUM") as ps_early:
            for pc in range(6):
                ps = ps_early.tile([128, L], F32, tag="xz", name="xz")
                for f0, fl in FCH:
                    for kc in range(2):
                        lhsT = (
                            winx[kc][:, pc * 128 : (pc + 1) * 128]
                            if pc < 4
                            else winz[kc][:, (pc - 4) * 128 : (pc - 3) * 128]
                        )
                        nc.tensor.matmul(
                            ps[:, f0 : f0 + fl], lhsT, hs[kc][:, f0 : f0 + fl],
                            start=(kc == 0), stop=(kc == 1),
                        )
                dst = xT[pc] if pc < 4 else zg[pc - 4]
                nc.scalar.activation(dst, ps, AF.Silu)

            # ---- x_proj -> x_dbl [48, L]; bounce B/C rows to DRAM ----
            xdbl = apool.tile([R + 2 * DS, L], BF16, tag="xdbl", name="xdbl")
            ps = ps_early.tile([R + 2 * DS, L], F32, tag="aux", name="aux")
            for f0, fl in FCH:
                for kc in range(4):
                    nc.tensor.matmul(
                        ps[:, f0 : f0 + fl], wx[kc], xT[kc][:, f0 : f0 + fl],
                        start=(kc == 0), stop=(kc == 3),
                    )
            nc.scalar.activation(xdbl, ps[0 : R + 2 * DS, :], AF.Copy)
            nc.sync.dma_start(out=bc_dram[:, :], in_=xdbl[R : R + 2 * DS, :])

            # ---- dt = softplus(dt_r @ WdtT + bdt), all under table 6 ----
            dtT = [apool.tile([128, L], BF16, tag=f"dtT{i}", name=f"dtT{i}") for i in range(NDT)]
            sp = [apool.tile([128, L], F32, tag=f"sp{i}", name=f"sp{i}") for i in range(NDT)]
            for dtc in range(NDT):
                ps = ps_early.tile([128, L], F32, tag="aux", name="aux")
                for f0, fl in FCH:
                    nc.tensor.matmul(
                        ps[:, f0 : f0 + fl],
                        wdt[0][:, dtc * 128 : (dtc + 1) * 128],
                        xdbl[0:R, f0 : f0 + fl], start=True, stop=True,
                    )
                nc.scalar.activation(sp[dtc], ps, AF.Exp, bias=bdt_s[:, dtc : dtc + 1])
            for dtc in range(NDT):
                nc.vector.tensor_scalar_add(sp[dtc], sp[dtc], 1.0)
            for dtc in range(NDT):
                nc.scalar.activation(dtT[dtc], sp[dtc], AF.Ln)

        # w2 = dt * x_own
        w2 = [apool.tile([128, L], BF16, tag=f"w2{i}", name=f"w2{i}") for i in range(NDT)]
        for dtc in range(NDT):
            nc.vector.tensor_mul(w2[dtc], dtT[dtc], xT[dtc])

        # ---- grouped scan loop ----
        ypsum = {}
        m_sb = apool.tile([128, 4], F32, tag="m", name="m")  # cols f0,f1,b0,b1
        yg = {}
        with tc.tile_pool(name="ps_y", bufs=1, space="PSUM") as ps_y, \
             tc.tile_pool(name="bcp", bufs=2) as bc_pool, \
             tc.tile_pool(name="dap", bufs=2 if DAF_BF16 else 1) as da_pool, \
             tc.tile_pool(name="dbp", bufs=2) as dbu_pool, \
             tc.tile_pool(name="hp", bufs=1) as h_pool:
            for dr in range(2):
                for dtc in range(NDT):
                    yp = ps_y.tile([128, L], F32, tag=f"y{dr}{dtc}", name=f"y{dr}{dtc}")
                    ypsum[(dr, dtc)] = yp
                    dd = (ddf if dr == 0 else ddb)[dtc]
                    for f0, fl in FCH:
                        nc.tensor.matmul(
                            yp[:, f0 : f0 + fl], dd, xT[dtc][:, f0 : f0 + fl],
                            start=True, stop=False, skip_group_check=True,
                        )

            for g in range(NG):
                n0 = g * G
                # B/C broadcast prefetch: B block then C block, one DMA each
                bcB = bc_pool.tile([128, G * L], BF16, tag="bcB", name="bcB")
                bcC = bc_pool.tile([128, G * L], BF16, tag="bcC", name="bcC")
                nc.sync.dma_start(
                    out=bcB,
                    in_=bass.AP(tensor=bc_dram, offset=n0 * L,
                                ap=[[0, 128], [L, G], [1, L]]),
                )
                nc.sync.dma_start(
                    out=bcC,
                    in_=bass.AP(tensor=bc_dram, offset=(DS + n0) * L,
                                ap=[[0, 128], [L, G], [1, L]]),
                )

                for dtc in range(NDT):
                    daf = da_pool.tile([128, GW], DAF_DT, tag=f"daf{dtc}", name=f"daf{dtc}")
                    for k in range(G):
                        col = dtc * DS + n0 + k
                        nc.scalar.activation(
                            _seg(daf, k), dtT[dtc], AF.Exp,
                            scale=af_s[:, col : col + 1],
                        )
                    if shared_a:
                        dab = daf
                    else:
                        dab = da_pool.tile([128, GW], DAF_DT, tag=f"dab{dtc}", name=f"dab{dtc}")
                        for k in range(G):
                            col = dtc * DS + n0 + k
                            nc.scalar.activation(
                                _seg(dab, k), dtT[dtc], AF.Exp,
                                scale=ab_s[:, col : col + 1],
                            )

                    # measured rates: DVE plain mult ~0.65us, Pool ~3.5us per
                    # [128,900]; Pool gets only what hides under the DVE scans
                    dbu = dbu_pool.tile([128, GW], BF16, tag=f"dbu{dtc}", name=f"dbu{dtc}")
                    for k in range(G):
                        pool_dbu = (dtc == 1 and k % 2 == 0) or (dtc == 0 and k == 0)
                        eng = nc.gpsimd if (DBU_POOL and pool_dbu) else nc.vector
                        eng.tensor_mul(_seg(dbu, k), w2[dtc],
                                       bcB[:, k * L : (k + 1) * L])

                    for dr in range(2):
                        h = h_pool.tile([128, GW], BF16, tag=f"h{dr}{dtc}", name=f"h{dr}{dtc}")
                        for k in range(G):
                            if dr == 0:
                                nc.vector.tensor_tensor_scan(
                                    _seg(h, k), _seg(daf, k), _seg(dbu, k),
                                    0.0, OP.mult, OP.add,
                                )
                            else:
                                nc.vector.tensor_tensor_scan(
                                    _seg(h, k)[:, ::-1], _seg(dab, k)[:, ::-1],
                                    _seg(dbu, k)[:, ::-1], 0.0, OP.mult, OP.add,
                                )
                            hc_eng = nc.gpsimd if (HC_POOL > 0 and dr == 1) else nc.vector
                            hc_eng.tensor_mul(_seg(h, k), _seg(h, k),
                                              bcC[:, k * L : (k + 1) * L])
                            yp = ypsum[(dr, dtc)]
                            for f0, fl in FCH:
                                nc.tensor.matmul(
                                    yp[:, f0 : f0 + fl],
                                    ident,
                                    _seg(h, k, f0, fl),
                                    start=False,
                                    stop=(g == NG - 1 and k == G - 1),
                                    skip_group_check=True,
                                )

            # ---- yg = y*zg; pooled sums via ACT accumulate ----
            for dr in range(2):
                for dtc in range(NDT):
                    c = 2 * dr + dtc
                    t = apool.tile([128, L], BF16, tag=f"yg{dr}{dtc}", name=f"yg{dr}{dtc}")
                    yg[(dr, dtc)] = t
                    nc.vector.tensor_mul(t, ypsum[(dr, dtc)], zg[dtc])
                    nc.scalar.activation(
                        t, t, AF.Copy, accum_out=m_sb[:, c : c + 1]
                    )

        # ---- exchange pooled vector; u2[p, j] = v_full[p + 128j] ----
        with tc.tile_pool(name="ps_tail", bufs=1, space="PSUM") as ps_tail:
            u2 = apool.tile([128, 8], F32, tag="u2", name="u2")
            if exch == "p2p":
                par = nc.sync.alloc_register("parity")
                nc.sync.reg_load(par, pct_s[0:1, 0:1])
                mflat = bass.AP(tensor=shm, offset=0, ap=[[1, 128], [128, 4]])
                mflat_hi = bass.AP(tensor=shm, offset=512, ap=[[1, 128], [128, 4]])
                with tc.If(nc.sync.snap(par) == 0) as pcmp:
                    nc.sync.dma_start(out=mflat, in_=m_sb).then_inc(psem, 16)
                    nc.sync.wait_ge(psem, 16)
                    nc.sync.dma_start(out=shfl[0:1, 0:4], in_=one_sb)
                with pcmp.Else():
                    nc.sync.dma_start(out=mflat_hi, in_=m_sb).then_inc(psem, 16)
                    nc.sync.wait_ge(psem, 16)
                    nc.sync.dma_start(out=shfl[0:1, 4:8], in_=one_sb)
                # bounded poll until BOTH flag words are set (mine is set by
                # queue order; the partner's is the real wait)
                flag_sb = apool.tile([1, 8], U32, tag="flag", name="flag")
                f0 = nc.sync.alloc_register("f0")
                f1 = nc.sync.alloc_register("f1")
                pwait = 32
                with ExitStack() as polls:
                    for _ in range(N_POLLS):
                        nc.sync.dma_start(out=flag_sb, in_=shfl[0:1, :]).then_inc(psem, 16)
                        nc.sync.wait_ge(psem, pwait)
                        pwait += 16
                        nc.sync.reg_load(f0, flag_sb[0:1, 0:1])
                        nc.sync.reg_load(f1, flag_sb[0:1, 4:5])
                        nc.sync.reg_alu(f0, nc.sync.snap(f0), nc.sync.snap(f1), OP.add)
                        polls.enter_context(tc.If(nc.sync.snap(f0) != 2))
                nc.sync.dma_start(
                    out=u2, in_=bass.AP(tensor=shm, offset=0, ap=[[1, 128], [128, 8]])
                )
            else:
                nc.gpsimd.dma_start(
                    out=bass.AP(tensor=u_cc_in, offset=0, ap=[[1, 128], [128, 4]]),
                    in_=m_sb,
                )
                nc.gpsimd.collective_compute(
                    "AllGather", OP.bypass,
                    replica_groups=[[0, 1], [2, 3], [4, 5], [6, 7]],
                    ins=[u_cc_in[:, :]], outs=[u_cc_out[:, :]],
                )
                nc.sync.dma_start(
                    out=u2,
                    in_=bass.AP(tensor=u_cc_out, offset=0, ap=[[1, 128], [128, 8]]),
                )

            # keep the PE p-state hot through the exchange wait: throwaway
            # matmuls on resident tiles (results copied to a dead scratch so
            # they survive DCE), so the gate/out-proj matmuls run at full clock
            warm = ps_tail.tile([128, 128], F32, tag="warm", name="warm")
            for wi in range(24):
                nc.tensor.matmul(warm, ident, xT[0][:, 0:128],
                                 start=True, stop=True, skip_group_check=True)
            wscr = apool.tile([1, 4], F32, tag="wscr", name="wscr")
            nc.scalar.activation(wscr, warm[0:1, 0:4], AF.Copy)
            nc.sync.dma_start(out=cc_zin[:, :], in_=wscr)

            # ---- gate: q = G2^T v + b; g = 1/(1+exp(-q)) ----
            u2b = apool.tile([128, 8], BF16, tag="u2b", name="u2b")
            nc.scalar.activation(u2b, u2, AF.Copy)
            vps = ps_tail.tile([1, 2 * DH], F32, tag="vps", name="vps")
            for kc in range(8):
                nc.tensor.matmul(
                    vps, u2b[:, kc : kc + 1], g2[kc], start=(kc == 0), stop=(kc == 7)
                )
            g_row = apool.tile([1, 2 * DH], F32, tag="grow", name="grow")
            e_row = apool.tile([1, 2 * DH], F32, tag="erow", name="erow")
            nc.vector.tensor_add(g_row, vps, bgate_r)
            nc.scalar.activation(e_row, g_row, AF.Exp, scale=-1.0)
            # bounce exp(-q) and finish the sigmoid in [128,4]: single-partition
            # [1,512] DVE ops are element-serial (reciprocal measured 3.3us)
            nc.sync.dma_start(out=g_dram[:, :], in_=e_row)
            e4 = apool.tile([128, 4], F32, tag="e4", name="e4")
            nc.sync.dma_start(
                out=e4,
                in_=bass.AP(tensor=g_dram, offset=0, ap=[[1, 128], [128, 4]]),
            )
            g_sb = apool.tile([128, 4], F32, tag="g", name="g")
            nc.vector.tensor_scalar_add(e4, e4, 1.0)
            nc.vector.reciprocal(g_sb, e4)

            for dr in range(2):
                for dtc in range(NDT):
                    c = 2 * dr + dtc
                    nc.vector.tensor_scalar_mul(
                        yg[(dr, dtc)], yg[(dr, dtc)], g_sb[:, c : c + 1]
                    )
            out_sb = [apool.tile([128, L], F32, tag=f"o{i}", name=f"o{i}")
                      for i in range(2)]
            for pc in range(2):
                ops_ = ps_tail.tile([128, L], F32, tag="ops", name="ops")
                for f0, fl in FCH:
                    for kc in range(4):
                        nc.tensor.matmul(
                            ops_[:, f0 : f0 + fl],
                            wo[kc][:, pc * 128 : (pc + 1) * 128],
                            yg[(kc // 2, kc % 2)][:, f0 : f0 + fl],
                            start=(kc == 0), stop=(kc == 3),
                        )
                nc.scalar.activation(out_sb[pc], ops_, AF.Copy)
                nc.sync.dma_start(
                    out=outp[pc * 128 : (pc + 1) * 128, :], in_=out_sb[pc]
                )

    nc.finalize()
    return nc


_NC_CACHE = {}


def _get_module(shared_a: bool, exch: str):
    key = (shared_a, exch)
    if key not in _NC_CACHE:
        _NC_CACHE[key] = _build_module(shared_a, exch)
    return _NC_CACHE[key]


def _diag_stack(d):
    out = np.zeros((DH, 128), dtype=np.float32)
    for t in range(NDT):
        out[t * 128 : (t + 1) * 128, :] = np.diag(d[t * 128 : (t + 1) * 128])
    return out


def kernel(**inputs):
    inp = {k: np.asarray(v, dtype=np.float32) for k, v in inputs.items()}
    hs = inp["hidden_states"]
    W_in, W_x, W_dt = inp["W_in"], inp["W_xproj"], inp["W_dt"]
    b_dt = inp["b_dt"]
    A_f = -np.exp(inp["A_log_f"])
    A_b = -np.exp(inp["A_log_b"])
    D_f, D_b = inp["D_f"], inp["D_b"]
    W_g, b_g = inp["W_global"], inp["b_global"]
    W_gate, b_gate = inp["W_gate"], inp["b_gate"]
    W_out = inp["W_out"]

    shared_a = bool(np.array_equal(A_f, A_b))
    I = np.eye(128, dtype=np.float32)
    bf = ml_dtypes.bfloat16
    in_maps = []
    for core in range(8):
        b, h = core // 2, core % 2
        o = h * DH
        perm = np.r_[o : o + DH, (DH - o) % DI : (DH - o) % DI + DH]
        ownc = np.r_[o : o + DH, DI + o : DI + o + DH]
        ccorder = np.r_[0:DH, DI : DI + DH, DH:DI, DI + DH : 2 * DI]

        def acol(A):
            a = A[o : o + DH].reshape(NDT, 128, DS)
            return np.ascontiguousarray(a.transpose(1, 0, 2).reshape(128, NDT * DS))

        m = {
            "hsT": np.ascontiguousarray(hs[b].T).astype(bf),
            "WinxT": np.ascontiguousarray(W_in[:DI][perm].T).astype(bf),
            "WinzT": np.ascontiguousarray(W_in[DI + o : DI + o + DH].T).astype(bf),
            "WxT": np.ascontiguousarray(W_x[:, perm].T).astype(bf),
            "WdtT": np.ascontiguousarray(W_dt[o : o + DH].T).astype(bf),
            "bdt": np.ascontiguousarray(b_dt[o : o + DH].reshape(NDT, 128).T),
            "Afc": acol(A_f),
            "Abc": acol(A_b),
            "Ddf": _diag_stack(D_f[o : o + DH]).astype(bf),
            "Ddb": _diag_stack(D_b[o : o + DH]).astype(bf),
            "I128": I.astype(bf),
            "G2T": np.ascontiguousarray(
                (W_gate[ownc] @ W_g[:, ccorder] / np.float32(L)).T
            ).astype(bf),
            "bgate2": np.ascontiguousarray(
                (b_gate[ownc] + W_gate[ownc] @ b_g).reshape(1, 512)
            ),
            "WoT": np.ascontiguousarray(W_out[:, ownc].T).astype(bf),
            "pctl": np.full((1, 4), h, np.uint32),
        }
        in_maps.append(m)

    nc = _get_module(shared_a, EXCH)
    res = run_bass_kernel_spmd(nc, in_maps, core_ids=list(range(8)))
    outs = res.results
    out = np.zeros((B, L, DM), dtype=np.float32)
    for b in range(B):
        part = outs[2 * b]["outp"] + outs[2 * b + 1]["outp"]
        out[b] = part.T
    return out

